# revision 4
# baseline (speedup 1.0000x reference)
"""Trainium2 Bass kernel for CrossAttentionAssociation.

Model: cross-attention (detections query tracks) + residual + LayerNorm,
then a pairwise association scorer:
  out[b,i,j] = sigmoid(w2 . relu(W1 (xn[b,i] * trk[b,j]) + b1) + b2)

Sharding (8 cores): core c handles batch b = c // 2 and detection rows
[256*(c%2), 256*(c%2)+256).  Tracks are replicated per batch.

v2 design (all-bf16 matmul path):
- host converts all weights/activations feeding PE to bf16 (no on-chip
  casts; FWL halves LDWEIGHTS cost for the per-i stationaries)
- attention per head in bf16, softmax without max-subtraction, fused
  denominator via ones-column; LayerNorm rstd via Ln+Exp so the whole
  pre-sigmoid kernel uses one ACT table set
- association per detection i: a_i = w1T*xn_i (DVE, bf16), H_i via 2
  accumulating bf16 matmuls into a 2-det [128,1024] PSUM tile;
  relu+b1 fused PSUM->SBUF on ACT or DVE (alternating, to split the
  elementwise floor across both engines); logits via shifted w2
  stationary with 4-way PE column tiling (4 concurrent M=32 matmuls),
  128 detections accumulate into one [128,512] PSUM tile; sigmoid+b2
  fused on ACT.
"""
import sys
import types

import numpy as np


def _install_ntff_hook():
    """Shim antenv.axon_hooks (absent on this image) so trace=True works."""
    if "antenv.axon_hooks" in sys.modules:
        return
    mod = types.ModuleType("antenv.axon_hooks")
    _hook = [None]
    mod.set_axon_ntff_profile_hook = lambda h: _hook.__setitem__(0, h)
    mod.get_axon_ntff_profile_hook = lambda: _hook[0]
    sys.modules["antenv.axon_hooks"] = mod
    try:
        from trn_agent_boot.trn_boot import _ntff_profile_via_ctypes
        mod.set_axon_ntff_profile_hook(
            _ntff_profile_via_ctypes("/opt/axon/libaxon_pjrt.so"))
    except Exception:
        pass


_install_ntff_hook()

import ml_dtypes  # noqa: E402
import concourse.bacc as bacc  # noqa: E402
import concourse.mybir as mybir  # noqa: E402
import concourse.tile as tile  # noqa: E402
from concourse.bass_utils import run_bass_kernel_spmd  # noqa: E402

F32 = mybir.dt.float32
BF16 = mybir.dt.bfloat16
AF = mybir.ActivationFunctionType
ALU = mybir.AluOpType
BF = ml_dtypes.bfloat16

B, ND, NT, D = 4, 512, 512, 256
H, DK = 8, 32
DHID = 128
NDC = 256          # detections per core
LN_EPS = 1e-5
N_CORES = 8

# relu engine split: of every 4 [128,1024] relu tiles, this many on ACT
RELU_ACT_OF4 = 3

_CACHE = {}


def _build():
    nc = bacc.Bacc("TRN2", target_bir_lowering=False, debug=False)

    def din(name, shape, dt=BF16):
        return nc.dram_tensor(name, shape, dt, kind="ExternalInput").ap()

    detT = din("detT", [D, NDC])            # det_chunk.T  bf16
    det_bo = din("det_bo", [NDC, D], F32)   # det_chunk + b_o  f32
    trkT = din("trkT", [D, NT])             # tracks[b].T  bf16
    wqT = din("wqT", [D, D])
    wkT = din("wkT", [D, D])
    wvT = din("wvT", [D, D])
    woT = din("woT", [D, D])
    bq = din("bq", [D], F32)
    bk = din("bk", [D], F32)
    bv = din("bv", [D], F32)
    lng = din("lng", [D], F32)
    lnb = din("lnb", [D], F32)
    w1T = din("w1T", [D, DHID])
    b1 = din("b1", [DHID], F32)
    w2s = din("w2s", [DHID, 32 * 32])       # shifted stationary blocks bf16
    b2b = din("b2b", [128], F32)
    ident = din("ident", [128, 128], F32)
    out = nc.dram_tensor("out", [NDC, NT], F32, kind="ExternalOutput").ap()

    with tile.TileContext(nc) as tc:
        with (
            tc.tile_pool(name="persist", bufs=1) as pp,
            tc.tile_pool(name="stage", bufs=1) as stg,
        ):
            # ---- load inputs (already bf16 from host; no casts) ----
            def load(ap, p, f, nt, dt=BF16):
                outs = []
                nm = ap.tensor.name
                for t in range(nt):
                    s = pp.tile([p, f], dt, tag=f"t_{nm}_{t}",
                                name=f"t_{nm}_{t}")
                    nc.sync.dma_start(s[:], ap[t * p:(t + 1) * p, :])
                    outs.append(s)
                return outs

            trkT_t = load(trkT, 128, NT, 2)
            detT_t = load(detT, 128, NDC, 2)
            wqT_t = load(wqT, 128, D, 2)
            wkT_t = load(wkT, 128, D, 2)
            wvT_t = load(wvT, 128, D, 2)
            woT_t = load(woT, 128, D, 2)
            w1T_t = load(w1T, 128, DHID, 2)
            w2s_t = load(w2s, 128, 32 * 32, 1)[0]
            det_bo_t = load(det_bo, 128, D, 2, F32)
            idn = load(ident, 128, 128, 1, F32)[0]

            def vec_tiles(ap, n=2):
                ts = []
                nm = ap.tensor.name
                for t in range(n):
                    s = pp.tile([128, 1], F32, tag=f"v_{nm}_{t}",
                                name=f"v_{nm}_{t}")
                    nc.sync.dma_start(s[:, 0], ap[t * 128:(t + 1) * 128])
                    ts.append(s)
                return ts

            bv_t = vec_tiles(bv)
            lng_t = vec_tiles(lng)
            lnb_t = vec_tiles(lnb)
            b1_t = vec_tiles(b1, 1)[0]
            b2_t = vec_tiles(b2b, 1)[0]
            bq_h = []
            bk_h = []
            for h in range(H):
                s = pp.tile([DK, 1], F32, tag=f"bq_{h}", name=f"bq_{h}")
                nc.sync.dma_start(s[:, 0], bq[h * DK:(h + 1) * DK])
                bq_h.append(s)
                s = pp.tile([DK, 1], F32, tag=f"bk_{h}", name=f"bk_{h}")
                nc.sync.dma_start(s[:, 0], bk[h * DK:(h + 1) * DK])
                bk_h.append(s)

            eps_t = pp.tile([128, 1], F32)
            nc.vector.memset(eps_t[:], LN_EPS)

            # ---- projections (per head, feature-major, bf16) ----
            qh = []   # [DK, NDC] bf16 per head
            kh = []   # [DK, NT] bf16 per head
            v_sb = []  # token-major V [128 j, 8*(32+2)] bf16 per j-chunk
            with tc.tile_pool(name="proj_ps", bufs=2, space="PSUM") as pps:
                for h in range(H):
                    ps = pps.tile([DK, NDC], F32, tag="q")
                    for dc in range(2):
                        nc.tensor.matmul(
                            ps[:], wqT_t[dc][:, h * DK:(h + 1) * DK],
                            detT_t[dc][:], start=(dc == 0), stop=(dc == 1))
                    q = pp.tile([DK, NDC], BF16, tag=f"qh_{h}",
                                name=f"qh_{h}")
                    nc.scalar.activation(q[:], ps[:], AF.Identity,
                                         bias=bq_h[h][:])
                    qh.append(q)

                    ps = pps.tile([DK, NT], F32, tag="k")
                    for dc in range(2):
                        nc.tensor.matmul(
                            ps[:], wkT_t[dc][:, h * DK:(h + 1) * DK],
                            trkT_t[dc][:], start=(dc == 0), stop=(dc == 1))
                    k = pp.tile([DK, NT], BF16, tag=f"kh_{h}",
                                name=f"kh_{h}")
                    nc.scalar.activation(k[:], ps[:], AF.Identity,
                                         bias=bk_h[h][:])
                    kh.append(k)

                ones8 = pp.tile([128, H], BF16)
                nc.vector.memset(ones8[:], 1.0)
                zero8 = pp.tile([128, H], BF16)
                nc.vector.memset(zero8[:], 0.0)
                for jc in range(4):
                    ps = pps.tile([128, D], F32, tag="v")
                    for dc in range(2):
                        nc.tensor.matmul(
                            ps[:], trkT_t[dc][:, jc * 128:(jc + 1) * 128],
                            wvT_t[dc][:], start=(dc == 0), stop=(dc == 1))
                    v = pp.tile([128, H * 34], BF16, tag=f"vsb_{jc}",
                                name=f"vsb_{jc}")
                    vr = v.rearrange("p (h c) -> p h c", c=34)
                    nc.vector.tensor_copy(
                        vr[:, :, 0:32], ps.rearrange("p (h c) -> p h c", c=32))
                    nc.vector.tensor_copy(
                        vr[:, :, 32:33],
                        ones8.rearrange("p (h o) -> p h o", o=1))
                    nc.vector.tensor_copy(
                        vr[:, :, 33:34],
                        zero8.rearrange("p (h o) -> p h o", o=1))
                    v_sb.append(v)

            # ---- attention: scores -> exp -> ctx/sums ----
            inv_sqrt_dk = 1.0 / np.sqrt(DK)
            with (
                tc.tile_pool(name="ctx_ps", bufs=1, space="PSUM") as cps,
                tc.tile_pool(name="eh_sb", bufs=3) as esb,
            ):
                psum_ctx = [cps.tile([128, H * 34], F32, tag=f"ctx{ic}",
                                     name=f"psum_ctx{ic}") for ic in range(2)]
                with tc.tile_pool(name="s_ps", bufs=2, space="PSUM") as sps:
                    for h in range(H):
                        for jc in range(4):
                            ps = sps.tile([128, NDC], F32, tag="s")
                            nc.tensor.matmul(
                                ps[:], kh[h][:, jc * 128:(jc + 1) * 128],
                                qh[h][:], start=True, stop=True)
                            e = esb.tile([128, NDC], BF16, tag=f"e{jc}")
                            nc.scalar.activation(e[:], ps[:], AF.Exp,
                                                 scale=inv_sqrt_dk)
                            for ic in range(2):
                                nc.tensor.matmul(
                                    psum_ctx[ic][:, h * 34:(h + 1) * 34],
                                    e[:, ic * 128:(ic + 1) * 128],
                                    v_sb[jc][:, h * 34:(h + 1) * 34],
                                    start=(jc == 0), stop=(jc == 3))

                # normalize ctx (token-major), transpose, +b_v
                recip = pp.tile([128, 2 * H], F32)
                for ic in range(2):
                    for h in range(H):
                        nc.vector.reciprocal(
                            recip[:, ic * H + h:ic * H + h + 1],
                            psum_ctx[ic][:, h * 34 + 32:h * 34 + 33])
                ctx_sb = []
                for ic in range(2):
                    c = pp.tile([128, D], F32, tag=f"ctx_sb_{ic}",
                                name=f"ctx_sb_{ic}")
                    for h in range(H):
                        nc.vector.tensor_scalar_mul(
                            c[:, h * DK:(h + 1) * DK],
                            psum_ctx[ic][:, h * 34:h * 34 + 32],
                            recip[:, ic * H + h:ic * H + h + 1])
                    ctx_sb.append(c)

            ctxT = [pp.tile([128, NDC], BF16, tag=f"ctxT{dc}",
                            name=f"ctxT{dc}") for dc in range(2)]
            xnT = [[pp.tile([128, 128], F32, tag=f"xnT{dc}_{ic}",
                            name=f"xnT{dc}_{ic}") for ic in range(2)]
                   for dc in range(2)]
            with tc.tile_pool(name="tr_ps", bufs=2, space="PSUM") as tps:
                for ic in range(2):
                    for dc in range(2):
                        pt = tps.tile([128, 128], F32, tag="tr")
                        nc.tensor.transpose(
                            pt[:], ctx_sb[ic][:, dc * 128:(dc + 1) * 128],
                            idn[:])
                        nc.scalar.activation(
                            ctxT[dc][:, ic * 128:(ic + 1) * 128], pt[:],
                            AF.Identity, bias=bv_t[dc][:])

                # ---- attended + residual + LayerNorm ----
                with tc.tile_pool(name="ln_ps", bufs=2, space="PSUM") as lps:
                    for ic in range(2):
                        ps = lps.tile([128, D], F32, tag="att")
                        for dc in range(2):
                            nc.tensor.matmul(
                                ps[:], ctxT[dc][:, ic * 128:(ic + 1) * 128],
                                woT_t[dc][:], start=(dc == 0), stop=(dc == 1))
                        x = stg.tile([128, D], F32, tag="x")
                        nc.vector.tensor_add(x[:], ps[:], det_bo_t[ic][:])
                        # stats: mean via reduce, E[x^2] via fused x*x+accum
                        ssum = stg.tile([128, 1], F32, tag="ssum")
                        nc.vector.reduce_sum(ssum[:], x[:],
                                             axis=mybir.AxisListType.X)
                        mu = stg.tile([128, 1], F32, tag="mu")
                        nc.vector.tensor_scalar_mul(mu[:], ssum[:], 1.0 / D)
                        sq = stg.tile([128, D], F32, tag="sq")
                        ssq = stg.tile([128, 1], F32, tag="ssq")
                        nc.vector.scalar_tensor_tensor(
                            sq[:], x[:], 1.0, x[:],
                            op0=ALU.mult, op1=ALU.mult, accum_out=ssq[:])
                        m2 = stg.tile([128, 1], F32, tag="m2")
                        nc.vector.tensor_scalar_mul(m2[:], ssq[:], 1.0 / D)
                        mu2 = stg.tile([128, 1], F32, tag="mu2")
                        nc.vector.tensor_mul(mu2[:], mu[:], mu[:])
                        var = stg.tile([128, 1], F32, tag="var")
                        nc.vector.tensor_sub(var[:], m2[:], mu2[:])
                        # rstd = exp(-0.5*ln(var+eps)) (stays in ln/exp set)
                        lv = stg.tile([128, 1], F32, tag="lv")
                        nc.scalar.activation(lv[:], var[:], AF.Ln,
                                             bias=eps_t[:])
                        rstd = stg.tile([128, 1], F32, tag="rstd")
                        nc.scalar.activation(rstd[:], lv[:], AF.Exp,
                                             scale=-0.5)
                        y = stg.tile([128, D], F32, tag="y")
                        nc.vector.tensor_scalar(
                            y[:], x[:], mu[:], rstd[:],
                            op0=ALU.subtract, op1=ALU.mult)
                        # transpose y, apply ln scale/shift feature-major
                        for dc in range(2):
                            pt = tps.tile([128, 128], F32, tag="tr")
                            nc.tensor.transpose(
                                pt[:], y[:, dc * 128:(dc + 1) * 128], idn[:])
                            nc.vector.tensor_scalar(
                                xnT[dc][ic][:], pt[:],
                                lng_t[dc][:], lnb_t[dc][:],
                                op0=ALU.mult, op1=ALU.add)

            # ---- association scorer ----
            # i = g*128 + 32*c + r; pairs (c0,c1)=(0,1),(2,3) share one
            # [128,1024] H psum tile; logits col-tiled 4-way into psum_l.
            with (
                tc.tile_pool(name="a_sb", bufs=6) as asb,
                tc.tile_pool(name="h_ps", bufs=3, space="PSUM") as hps,
                tc.tile_pool(name="rt_sb", bufs=4) as rsb,
                tc.tile_pool(name="l_ps", bufs=2, space="PSUM") as lqs,
                tc.tile_pool(name="sig_sb", bufs=2) as ssb,
            ):
                pair_idx = 0
                for g in range(NDC // 128):
                    psum_l = lqs.tile([128, NT], F32, tag="l")
                    for r in range(32):
                        for c0 in (0, 2):
                            ph = hps.tile([128, 2 * NT], F32, tag="h")
                            rt = rsb.tile([128, 2 * NT], BF16, tag="r")
                            for k in range(2):
                                i = g * 128 + 32 * (c0 + k) + r
                                ic, col = divmod(i, 128)
                                a = asb.tile([128, 2 * DHID], BF16, tag="a")
                                nc.vector.tensor_scalar_mul(
                                    a[:, 0:DHID], w1T_t[0][:],
                                    xnT[0][ic][:, col:col + 1])
                                nc.vector.tensor_scalar_mul(
                                    a[:, DHID:2 * DHID], w1T_t[1][:],
                                    xnT[1][ic][:, col:col + 1])
                                nc.tensor.matmul(
                                    ph[:, k * NT:(k + 1) * NT],
                                    a[:, 0:DHID], trkT_t[0][:],
                                    start=True, stop=False)
                                nc.tensor.matmul(
                                    ph[:, k * NT:(k + 1) * NT],
                                    a[:, DHID:2 * DHID], trkT_t[1][:],
                                    start=False, stop=True)
                            # relu + b1, PSUM->SBUF, alternating engine
                            if pair_idx % 4 < RELU_ACT_OF4:
                                nc.scalar.activation(rt[:], ph[:], AF.Relu,
                                                     bias=b1_t[:])
                            else:
                                nc.vector.tensor_scalar(
                                    rt[:], ph[:], b1_t[:], 0.0,
                                    op0=ALU.add, op1=ALU.max)
                            pair_idx += 1
                            for k in range(2):
                                c = c0 + k
                                nc.tensor.matmul(
                                    psum_l[32 * c:32 * c + 32, :],
                                    w2s_t[:, r * 32:(r + 1) * 32],
                                    rt[:, k * NT:(k + 1) * NT],
                                    start=(r == 0), stop=(r == 31),
                                    tile_position=(0, 32 * c))
                    sg = ssb.tile([128, NT], F32, tag="sig")
                    nc.scalar.activation(sg[:], psum_l[:], AF.Sigmoid,
                                         bias=b2_t[:])
                    nc.sync.dma_start(
                        out[g * 128:(g + 1) * 128, :], sg[:])

    nc.compile()
    return nc


def _host_prep(inputs):
    """Build the 8 per-core input maps from full inputs (numpy, cheap)."""
    det = np.ascontiguousarray(inputs["detections"], np.float32)
    trk = np.ascontiguousarray(inputs["tracks"], np.float32)
    f32 = lambda x: np.ascontiguousarray(np.asarray(x), np.float32)
    bf = lambda x: np.ascontiguousarray(np.asarray(x, np.float32)).astype(BF)
    w_q, b_q = f32(inputs["w_q"]), f32(inputs["b_q"])
    w_k, b_k = f32(inputs["w_k"]), f32(inputs["b_k"])
    w_v, b_v = f32(inputs["w_v"]), f32(inputs["b_v"])
    w_o, b_o = f32(inputs["w_o"]), f32(inputs["b_o"])
    ln_g, ln_b = f32(inputs["ln_g"]), f32(inputs["ln_b"])
    w1, b1 = f32(inputs["w1"]), f32(inputs["b1"])
    w2, b2 = f32(inputs["w2"]), f32(inputs["b2"])

    w2s = np.zeros((DHID, 32 * 32), np.float32)
    for r in range(32):
        w2s[:, r * 32 + r] = w2[0]
    shared = {
        "wqT": bf(w_q.T), "wkT": bf(w_k.T),
        "wvT": bf(w_v.T), "woT": bf(w_o.T),
        "bq": b_q, "bk": b_k, "bv": b_v,
        "lng": ln_g, "lnb": ln_b,
        "w1T": bf(w1.T), "b1": b1,
        "w2s": bf(w2s), "b2b": np.full(128, b2[0], np.float32),
        "ident": np.eye(128, dtype=np.float32),
    }
    in_maps = []
    for c in range(N_CORES):
        b, half = divmod(c, 2)
        dchunk = det[b, half * NDC:(half + 1) * NDC, :]
        m = dict(shared)
        m["detT"] = bf(dchunk.T)
        m["det_bo"] = np.ascontiguousarray(dchunk + b_o[None, :])
        m["trkT"] = bf(trk[b].T)
        in_maps.append(m)
    return in_maps


def _get_nc():
    if "nc" not in _CACHE:
        _CACHE["nc"] = _build()
    return _CACHE["nc"]


def run(inputs, trace=False):
    nc = _get_nc()
    in_maps = _host_prep(inputs)
    res = run_bass_kernel_spmd(nc, in_maps, core_ids=list(range(N_CORES)),
                               trace=trace)
    full = np.empty((B, ND, NT), np.float32)
    for c in range(N_CORES):
        b, half = divmod(c, 2)
        full[b, half * NDC:(half + 1) * NDC, :] = res.results[c]["out"]
    return full, res


def kernel(**inputs):
    return run(inputs, trace=False)[0]


# revision 16
# speedup vs baseline: 1.2279x; 1.2279x over previous
"""Trainium2 Bass kernel for CrossAttentionAssociation.

Model: cross-attention (detections query tracks) + residual + LayerNorm,
then a pairwise association scorer:
  out[b,i,j] = sigmoid(w2 . relu(W1 (xn[b,i] * trk[b,j]) + b1) + b2)

Sharding (8 cores): core c handles batch b = c // 2 and detection rows
[256*(c%2), 256*(c%2)+256).  Tracks are replicated per batch.

v2 design (all-bf16 matmul path):
- host converts all weights/activations feeding PE to bf16 (no on-chip
  casts; FWL halves LDWEIGHTS cost for the per-i stationaries)
- attention per head in bf16, softmax without max-subtraction, fused
  denominator via ones-column; LayerNorm rstd via Ln+Exp so the whole
  pre-sigmoid kernel uses one ACT table set
- association per detection i: a_i = w1T*xn_i (DVE, bf16), H_i via 2
  accumulating bf16 matmuls into a 2-det [128,1024] PSUM tile;
  relu+b1 fused PSUM->SBUF on ACT or DVE (alternating, to split the
  elementwise floor across both engines); logits via shifted w2
  stationary with 4-way PE column tiling (4 concurrent M=32 matmuls),
  128 detections accumulate into one [128,512] PSUM tile; sigmoid+b2
  fused on ACT.
"""
import sys
import types

import numpy as np


def _install_ntff_hook():
    """Shim antenv.axon_hooks (absent on this image) so trace=True works."""
    if "antenv.axon_hooks" in sys.modules:
        return
    mod = types.ModuleType("antenv.axon_hooks")
    _hook = [None]
    mod.set_axon_ntff_profile_hook = lambda h: _hook.__setitem__(0, h)
    mod.get_axon_ntff_profile_hook = lambda: _hook[0]
    sys.modules["antenv.axon_hooks"] = mod
    try:
        from trn_agent_boot.trn_boot import _ntff_profile_via_ctypes
        mod.set_axon_ntff_profile_hook(
            _ntff_profile_via_ctypes("/opt/axon/libaxon_pjrt.so"))
    except Exception:
        pass


_install_ntff_hook()

import ml_dtypes  # noqa: E402
import concourse.bacc as bacc  # noqa: E402
import concourse.mybir as mybir  # noqa: E402
import concourse.tile as tile  # noqa: E402
from concourse.bass_utils import run_bass_kernel_spmd  # noqa: E402

F32 = mybir.dt.float32
BF16 = mybir.dt.bfloat16
AF = mybir.ActivationFunctionType
ALU = mybir.AluOpType
BF = ml_dtypes.bfloat16

B, ND, NT, D = 4, 512, 512, 256
H, DK = 8, 32
DHID = 128
NDC = 256          # detections per core
LN_EPS = 1e-5
N_CORES = 8

# relu engine split: of every 4 [128,1024] relu tiles, this many on ACT
RELU_ACT_OF4 = 3

_CACHE = {}


def _build():
    nc = bacc.Bacc("TRN2", target_bir_lowering=False, debug=False)

    def din(name, shape, dt=BF16):
        return nc.dram_tensor(name, shape, dt, kind="ExternalInput").ap()

    detT = din("detT", [D, NDC])            # det_chunk.T  bf16
    det_bo = din("det_bo", [NDC, D])        # det_chunk + b_o  bf16
    trkT = din("trkT", [D, NT])             # tracks[b].T  bf16
    wqT = din("wqT", [D, D])
    wkT = din("wkT", [D, D])
    wvT = din("wvT", [D, D])
    woT = din("woT", [D, D])
    bq = din("bq", [D], F32)
    bk = din("bk", [D], F32)
    bv = din("bv", [D], F32)
    lng = din("lng", [D], F32)
    lnb = din("lnb", [D], F32)
    w1T = din("w1T", [D, DHID])
    b1 = din("b1", [DHID], F32)
    w2s = din("w2s", [DHID, 32 * 32])       # shifted stationary blocks bf16
    b2b = din("b2b", [128], F32)
    ident = din("ident", [128, 128], F32)
    out = nc.dram_tensor("out", [NDC, NT], F32, kind="ExternalOutput").ap()

    with tile.TileContext(nc) as tc:
        with (
            tc.tile_pool(name="persist", bufs=1) as pp,
            tc.tile_pool(name="stage", bufs=1) as stg,
        ):
            # ---- load inputs (already bf16 from host; no casts) ----
            def load(ap, p, f, nt, dt=BF16):
                outs = []
                nm = ap.tensor.name
                for t in range(nt):
                    s = pp.tile([p, f], dt, tag=f"t_{nm}_{t}",
                                name=f"t_{nm}_{t}")
                    nc.sync.dma_start(s[:], ap[t * p:(t + 1) * p, :])
                    outs.append(s)
                return outs

            wqT_t = load(wqT, 128, D, 2)
            detT_t = load(detT, 128, NDC, 2)
            wkT_t = load(wkT, 128, D, 2)
            trkT_t = load(trkT, 128, NT, 2)
            wvT_t = load(wvT, 128, D, 2)
            woT_t = load(woT, 128, D, 2)
            w1T_t = load(w1T, 128, DHID, 2)
            w2s_t = load(w2s, 128, 32 * 32, 1)[0]
            det_bo_t = load(det_bo, 128, D, 2)
            idn = load(ident, 128, 128, 1, F32)[0]

            def vec_tiles(ap, n=2):
                ts = []
                nm = ap.tensor.name
                for t in range(n):
                    s = pp.tile([128, 1], F32, tag=f"v_{nm}_{t}",
                                name=f"v_{nm}_{t}")
                    nc.sync.dma_start(s[:, 0], ap[t * 128:(t + 1) * 128])
                    ts.append(s)
                return ts

            bv_t = vec_tiles(bv)
            lng_t = vec_tiles(lng)
            lnb_t = vec_tiles(lnb)
            b1_t = vec_tiles(b1, 1)[0]
            b2_t = vec_tiles(b2b, 1)[0]
            bq_h = []
            bk_h = []
            for h in range(H):
                s = pp.tile([DK, 1], F32, tag=f"bq_{h}", name=f"bq_{h}")
                nc.sync.dma_start(s[:, 0], bq[h * DK:(h + 1) * DK])
                bq_h.append(s)
                s = pp.tile([DK, 1], F32, tag=f"bk_{h}", name=f"bk_{h}")
                nc.sync.dma_start(s[:, 0], bk[h * DK:(h + 1) * DK])
                bk_h.append(s)

            eps_t = pp.tile([128, 1], F32)
            nc.vector.memset(eps_t[:], LN_EPS)

            # ---- projections (per head, feature-major, bf16) ----
            qh = []   # [DK, NDC] bf16 per head
            kh = []   # [DK, NT] bf16 per head
            v_sb = []  # token-major V [128 j, 8*(32+2)] bf16 per j-chunk
            with tc.tile_pool(name="proj_ps", bufs=2, space="PSUM") as pps:
                for h in range(H):
                    ps = pps.tile([DK, NDC], F32, tag="q")
                    for dc in range(2):
                        nc.tensor.matmul(
                            ps[:], wqT_t[dc][:, h * DK:(h + 1) * DK],
                            detT_t[dc][:], start=(dc == 0), stop=(dc == 1))
                    q = pp.tile([DK, NDC], BF16, tag=f"qh_{h}",
                                name=f"qh_{h}")
                    nc.vector.tensor_scalar(q[:], ps[:], bq_h[h][:], 0.0,
                                            op0=ALU.add, op1=ALU.add)
                    qh.append(q)

                    ps = pps.tile([DK, NT], F32, tag="k")
                    for dc in range(2):
                        nc.tensor.matmul(
                            ps[:], wkT_t[dc][:, h * DK:(h + 1) * DK],
                            trkT_t[dc][:], start=(dc == 0), stop=(dc == 1))
                    k = pp.tile([DK, NT], BF16, tag=f"kh_{h}",
                                name=f"kh_{h}")
                    nc.vector.tensor_scalar(k[:], ps[:], bk_h[h][:], 0.0,
                                            op0=ALU.add, op1=ALU.add)
                    kh.append(k)

                ones8 = pp.tile([128, H], BF16)
                nc.vector.memset(ones8[:], 1.0)
                zero8 = pp.tile([128, H], BF16)
                nc.vector.memset(zero8[:], 0.0)
                for jc in range(4):
                    ps = pps.tile([128, D], F32, tag="v")
                    for dc in range(2):
                        nc.tensor.matmul(
                            ps[:], trkT_t[dc][:, jc * 128:(jc + 1) * 128],
                            wvT_t[dc][:], start=(dc == 0), stop=(dc == 1))
                    v = pp.tile([128, H * 34], BF16, tag=f"vsb_{jc}",
                                name=f"vsb_{jc}")
                    vr = v.rearrange("p (h c) -> p h c", c=34)
                    nc.vector.tensor_copy(
                        vr[:, :, 0:32], ps.rearrange("p (h c) -> p h c", c=32))
                    nc.vector.tensor_copy(
                        vr[:, :, 32:33],
                        ones8.rearrange("p (h o) -> p h o", o=1))
                    nc.vector.tensor_copy(
                        vr[:, :, 33:34],
                        zero8.rearrange("p (h o) -> p h o", o=1))
                    v_sb.append(v)

            # ---- attention: scores -> exp -> ctx/sums ----
            inv_sqrt_dk = 1.0 / np.sqrt(DK)
            with (
                tc.tile_pool(name="ctx_ps", bufs=1, space="PSUM") as cps,
                tc.tile_pool(name="eh_sb", bufs=3) as esb,
            ):
                psum_ctx = [cps.tile([128, H * 34], F32, tag=f"ctx{ic}",
                                     name=f"psum_ctx{ic}") for ic in range(2)]
                with tc.tile_pool(name="s_ps", bufs=2, space="PSUM") as sps:
                    # scores for 4 heads share one 2-bank psum tile and
                    # one Exp instruction
                    for jc in range(4):
                        for hh in (0, 4):
                            s4 = sps.tile([128, 4 * NDC], F32, tag="s4")
                            for h4 in range(4):
                                h = hh + h4
                                nc.tensor.matmul(
                                    s4[:, h4 * NDC:(h4 + 1) * NDC],
                                    kh[h][:, jc * 128:(jc + 1) * 128],
                                    qh[h][:], start=True, stop=True)
                            e = esb.tile([128, 4 * NDC], BF16, tag="e4")
                            nc.scalar.activation(e[:], s4[:], AF.Exp,
                                                 scale=inv_sqrt_dk)
                            for h4 in range(4):
                                h = hh + h4
                                for ic in range(2):
                                    nc.tensor.matmul(
                                        psum_ctx[ic][:, h * 34:(h + 1) * 34],
                                        e[:, h4 * NDC + ic * 128:
                                           h4 * NDC + (ic + 1) * 128],
                                        v_sb[jc][:, h * 34:(h + 1) * 34],
                                        start=(jc == 0), stop=(jc == 3))

                # normalize ctx (token-major), transpose, +b_v
                recip = pp.tile([128, 2 * H], F32)
                for ic in range(2):
                    for h in range(H):
                        nc.vector.reciprocal(
                            recip[:, ic * H + h:ic * H + h + 1],
                            psum_ctx[ic][:, h * 34 + 32:h * 34 + 33])
                ctx_sb = []
                for ic in range(2):
                    c = pp.tile([128, D], F32, tag=f"ctx_sb_{ic}",
                                name=f"ctx_sb_{ic}")
                    for h in range(H):
                        nc.vector.tensor_scalar_mul(
                            c[:, h * DK:(h + 1) * DK],
                            psum_ctx[ic][:, h * 34:h * 34 + 32],
                            recip[:, ic * H + h:ic * H + h + 1])
                    ctx_sb.append(c)

            ctxT = [pp.tile([128, NDC], BF16, tag=f"ctxT{dc}",
                            name=f"ctxT{dc}") for dc in range(2)]
            xnT = [[pp.tile([128, 128], F32, tag=f"xnT{dc}_{ic}",
                            name=f"xnT{dc}_{ic}") for ic in range(2)]
                   for dc in range(2)]
            with tc.tile_pool(name="tr_ps", bufs=2, space="PSUM") as tps:
                for ic in range(2):
                    for dc in range(2):
                        pt = tps.tile([128, 128], F32, tag="tr")
                        nc.tensor.transpose(
                            pt[:], ctx_sb[ic][:, dc * 128:(dc + 1) * 128],
                            idn[:])
                        nc.scalar.activation(
                            ctxT[dc][:, ic * 128:(ic + 1) * 128], pt[:],
                            AF.Identity, bias=bv_t[dc][:])

                # ---- attended + residual + LayerNorm ----
                with tc.tile_pool(name="ln_ps", bufs=2, space="PSUM") as lps:
                    for ic in range(2):
                        ps = lps.tile([128, D], F32, tag="att")
                        for dc in range(2):
                            nc.tensor.matmul(
                                ps[:], ctxT[dc][:, ic * 128:(ic + 1) * 128],
                                woT_t[dc][:], start=(dc == 0), stop=(dc == 1))
                        x = stg.tile([128, D], F32, tag="x")
                        nc.vector.tensor_add(x[:], ps[:], det_bo_t[ic][:])
                        # stats: mean via reduce, E[x^2] via fused x*x+accum
                        ssum = stg.tile([128, 1], F32, tag="ssum")
                        nc.vector.reduce_sum(ssum[:], x[:],
                                             axis=mybir.AxisListType.X)
                        mu = stg.tile([128, 1], F32, tag="mu")
                        nc.vector.tensor_scalar_mul(mu[:], ssum[:], 1.0 / D)
                        sq = stg.tile([128, D], F32, tag="sq")
                        ssq = stg.tile([128, 1], F32, tag="ssq")
                        nc.vector.scalar_tensor_tensor(
                            sq[:], x[:], 1.0, x[:],
                            op0=ALU.mult, op1=ALU.mult, accum_out=ssq[:])
                        m2 = stg.tile([128, 1], F32, tag="m2")
                        nc.vector.tensor_scalar_mul(m2[:], ssq[:], 1.0 / D)
                        mu2 = stg.tile([128, 1], F32, tag="mu2")
                        nc.vector.tensor_mul(mu2[:], mu[:], mu[:])
                        var = stg.tile([128, 1], F32, tag="var")
                        nc.vector.tensor_sub(var[:], m2[:], mu2[:])
                        sd = stg.tile([128, 1], F32, tag="sd")
                        nc.scalar.activation(sd[:], var[:], AF.Sqrt,
                                             bias=eps_t[:])
                        rstd = stg.tile([128, 1], F32, tag="rstd")
                        nc.vector.reciprocal(rstd[:], sd[:])
                        y = stg.tile([128, D], F32, tag="y")
                        nc.vector.tensor_scalar(
                            y[:], x[:], mu[:], rstd[:],
                            op0=ALU.subtract, op1=ALU.mult)
                        # transpose y, apply ln scale/shift feature-major
                        for dc in range(2):
                            pt = tps.tile([128, 128], F32, tag="tr")
                            nc.tensor.transpose(
                                pt[:], y[:, dc * 128:(dc + 1) * 128], idn[:])
                            nc.vector.tensor_scalar(
                                xnT[dc][ic][:], pt[:],
                                lng_t[dc][:], lnb_t[dc][:],
                                op0=ALU.mult, op1=ALU.add)

            # ---- association scorer ----
            # i = g*128 + 32*c + r; pairs (c0,c1)=(0,1),(2,3) share one
            # [128,1024] H psum tile; logits col-tiled 4-way into psum_l.
            with (
                tc.tile_pool(name="a_sb", bufs=6) as asb,
                tc.tile_pool(name="h_ps", bufs=3, space="PSUM") as hps,
                tc.tile_pool(name="rt_sb", bufs=4) as rsb,
                tc.tile_pool(name="l_ps", bufs=2, space="PSUM") as lqs,
                tc.tile_pool(name="sig_sb", bufs=2) as ssb,
            ):
                pair_idx = 0
                for g in range(NDC // 128):
                    psum_l = lqs.tile([128, NT], F32, tag="l")
                    for r in range(32):
                        rts = []
                        for c0 in (0, 2):
                            ph = hps.tile([128, 2 * NT], F32, tag="h")
                            rt = rsb.tile([128, 2 * NT], BF16, tag="r")
                            for k in range(2):
                                i = g * 128 + 32 * (c0 + k) + r
                                ic, col = divmod(i, 128)
                                a = asb.tile([128, 2 * DHID], BF16, tag="a")
                                nc.vector.tensor_scalar_mul(
                                    a[:, 0:DHID], w1T_t[0][:],
                                    xnT[0][ic][:, col:col + 1])
                                nc.vector.tensor_scalar_mul(
                                    a[:, DHID:2 * DHID], w1T_t[1][:],
                                    xnT[1][ic][:, col:col + 1])
                                nc.tensor.matmul(
                                    ph[:, k * NT:(k + 1) * NT],
                                    a[:, 0:DHID], trkT_t[0][:],
                                    start=True, stop=False)
                                nc.tensor.matmul(
                                    ph[:, k * NT:(k + 1) * NT],
                                    a[:, DHID:2 * DHID], trkT_t[1][:],
                                    start=False, stop=True)
                            # relu + b1, PSUM->SBUF, alternating engine
                            if pair_idx % 4 < RELU_ACT_OF4:
                                nc.scalar.activation(rt[:], ph[:], AF.Relu,
                                                     bias=b1_t[:])
                            else:
                                nc.vector.tensor_scalar(
                                    rt[:], ph[:], b1_t[:], 0.0,
                                    op0=ALU.add, op1=ALU.max)
                            pair_idx += 1
                            rts.append(rt)
                        # all four col-tiled logits matmuls back-to-back so
                        # the 4 col-groups run concurrently in the array
                        for c in range(4):
                            nc.tensor.matmul(
                                psum_l[32 * c:32 * c + 32, :],
                                w2s_t[:, r * 32:(r + 1) * 32],
                                rts[c // 2][:, (c % 2) * NT:(c % 2 + 1) * NT],
                                start=(r == 0), stop=(r == 31),
                                tile_position=(0, 32 * c))
                    sg = ssb.tile([128, NT], F32, tag="sig")
                    nc.scalar.activation(sg[:], psum_l[:], AF.Sigmoid,
                                         bias=b2_t[:])
                    nc.sync.dma_start(
                        out[g * 128:(g + 1) * 128, :], sg[:])

    nc.compile()
    return nc


def _host_prep(inputs):
    """Build the 8 per-core input maps from full inputs (numpy, cheap)."""
    det = np.ascontiguousarray(inputs["detections"], np.float32)
    trk = np.ascontiguousarray(inputs["tracks"], np.float32)
    f32 = lambda x: np.ascontiguousarray(np.asarray(x), np.float32)
    bf = lambda x: np.ascontiguousarray(np.asarray(x, np.float32)).astype(BF)
    w_q, b_q = f32(inputs["w_q"]), f32(inputs["b_q"])
    w_k, b_k = f32(inputs["w_k"]), f32(inputs["b_k"])
    w_v, b_v = f32(inputs["w_v"]), f32(inputs["b_v"])
    w_o, b_o = f32(inputs["w_o"]), f32(inputs["b_o"])
    ln_g, ln_b = f32(inputs["ln_g"]), f32(inputs["ln_b"])
    w1, b1 = f32(inputs["w1"]), f32(inputs["b1"])
    w2, b2 = f32(inputs["w2"]), f32(inputs["b2"])

    w2s = np.zeros((DHID, 32 * 32), np.float32)
    for r in range(32):
        w2s[:, r * 32 + r] = w2[0]
    shared = {
        "wqT": bf(w_q.T), "wkT": bf(w_k.T),
        "wvT": bf(w_v.T), "woT": bf(w_o.T),
        "bq": b_q, "bk": b_k, "bv": b_v,
        "lng": ln_g, "lnb": ln_b,
        "w1T": bf(w1.T), "b1": b1,
        "w2s": bf(w2s), "b2b": np.full(128, b2[0], np.float32),
        "ident": np.eye(128, dtype=np.float32),
    }
    in_maps = []
    for c in range(N_CORES):
        b, half = divmod(c, 2)
        dchunk = det[b, half * NDC:(half + 1) * NDC, :]
        m = dict(shared)
        m["detT"] = bf(dchunk.T)
        m["det_bo"] = bf(dchunk + b_o[None, :])
        m["trkT"] = bf(trk[b].T)
        in_maps.append(m)
    return in_maps


def _get_nc():
    if "nc" not in _CACHE:
        _CACHE["nc"] = _build()
    return _CACHE["nc"]


def run(inputs, trace=False):
    nc = _get_nc()
    in_maps = _host_prep(inputs)
    res = run_bass_kernel_spmd(nc, in_maps, core_ids=list(range(N_CORES)),
                               trace=trace)
    full = np.empty((B, ND, NT), np.float32)
    for c in range(N_CORES):
        b, half = divmod(c, 2)
        full[b, half * NDC:(half + 1) * NDC, :] = res.results[c]["out"]
    return full, res


def kernel(**inputs):
    return run(inputs, trace=False)[0]
